# revision 21
# baseline (speedup 1.0000x reference)
"""Trainium2 Bass kernel for MinibatchDiscrimination.

Reference computation:
    M = (x @ T).reshape(B, OUT_F, INTER_F)              # [128, 128, 32]
    l1[i,j,o] = sum_k |M[i,o,k] - M[j,o,k]|             # [128, 128, 128]
    o_b = sum_j exp(-l1) - 1                            # [128, 128]
    out = concat([x, o_b], axis=1)                      # [128, 1152]

Sharding: each of the 8 cores owns 16 of the 128 output features (o).
Per core, for each o the pairwise difference tensor
    D[i, (j,k)] = M[i,o,k] - M[j,o,k]
is produced by K=33 TensorEngine matmuls:
    lhsT  [33, 128]: rows 0..31 = M_o^T (row c, col i = M[i,o,c]), row 32 = 1
    rhs   [33, 4096]: rows 0..31 = BlockOnes (delta(c==k) per (j,k) col),
                      row 32     = vec(-M_o) flattened j-major
    out[i, 32j+k] = M[i,o,k]*1 - M[j,o,k]
The VectorEngine folds abs+sum-over-k in one op straight out of PSUM
(tensor_reduce(apply_absolute_value=True)), and the ScalarEngine computes
exp(-l1) with a fused accumulate over j (activation accum_out).  The
diagonal term exp(0) is computed by the same ACT path on a zero input and
subtracted, so it cancels exactly.

The per-o lhsT tiles (M_o^T plus a built-in ones row) are each computed
directly on the PE as T_ext_o^T @ x_ext, where host-prepped T_ext carries
a one-hot column and x_ext a ones row, so no cross-partition copies are
needed and every PE instruction carries at most 2 semaphore waits (HW
limit).

The x-passthrough part of the output is done on host.
"""

import numpy as np

B = 128
IN_F = 1024
OUT_F = 128
INTER_F = 32
N_CORES = 8
O_PER_CORE = OUT_F // N_CORES  # 16 output features per core
COLS_PER_CORE = O_PER_CORE * INTER_F  # 512 columns of T per core
PAIR_COLS = B * INTER_F  # 4096 = (j, k) flattened
KE = IN_F + 128  # padded contraction: 1024 (+ ones row at 1024, zeros after)
GW = INTER_F + 1  # 33: group width in T_ext (32 T columns + one-hot col)

_cache = {}


def _build_bass():
    import concourse.bass as bass
    import concourse.bacc as bacc
    import concourse.tile as tile
    import concourse.mybir as mybir

    fp32 = mybir.dt.float32
    bf16 = mybir.dt.bfloat16

    nc = bacc.Bacc("TRN2")

    xe_in = nc.dram_tensor("xe", [KE, B], bf16, kind="ExternalInput")
    te_in = nc.dram_tensor("te", [KE, O_PER_CORE * GW], bf16, kind="ExternalInput")
    bones_in = nc.dram_tensor("bones", [INTER_F, PAIR_COLS], bf16, kind="ExternalInput")
    ob_out = nc.dram_tensor("ob", [B, O_PER_CORE], fp32, kind="ExternalOutput")

    KK = KE // 128  # 9 contraction tiles

    with tile.TileContext(nc) as tc:
        with (
            tc.tile_pool(name="const", bufs=1) as const_pool,
            tc.tile_pool(name="work", bufs=2) as work_pool,
            tc.tile_pool(name="psum", bufs=2, space="PSUM") as psum_pool,
            tc.tile_pool(name="psumd", bufs=2, space="PSUM") as psumd_pool,
            tc.tile_pool(name="psumt", bufs=1, space="PSUM") as psumt_pool,
        ):
            # ---- load inputs ----
            xe_tiles = []
            for kk in range(KK):
                t = const_pool.tile([128, B], bf16, tag=f"xe{kk}")
                nc.sync.dma_start(t[:], xe_in[kk * 128 : (kk + 1) * 128, :])
                xe_tiles.append(t)
            te_tiles = []
            for kk in range(KK):
                t = const_pool.tile([128, O_PER_CORE * GW], bf16, tag=f"te{kk}")
                nc.sync.dma_start(t[:], te_in[kk * 128 : (kk + 1) * 128, :])
                te_tiles.append(t)

            # single rhs slot [33, 4096]: rows 0..31 = BlockOnes, row 32 per-o
            slot = const_pool.tile([GW, PAIR_COLS], bf16, tag="slot")
            nc.sync.dma_start(slot[0:INTER_F, :], bones_in[:])

            # ---- stage 1a: M = x @ T_c -> PSUM [128 (i), 512 (o,k)] ----
            # rhs: T columns of each 33-group, skipping the one-hot column
            ps_m = psum_pool.tile([128, COLS_PER_CORE], fp32, tag="ps")
            for kk in range(KK - 1):
                te3 = te_tiles[kk][:].rearrange("p (o c) -> p o c", c=GW)
                nc.tensor.matmul(
                    ps_m[:],
                    lhsT=xe_tiles[kk][:],
                    rhs=te3[:, :, 0:INTER_F],
                    start=(kk == 0),
                    stop=(kk == KK - 2),
                )
            m_neg = const_pool.tile([128, COLS_PER_CORE], bf16, tag="m_neg")
            nc.vector.tensor_scalar_mul(m_neg[:], ps_m[:], -1.0)

            # observer matmul: makes PE wait once on the bones-load semaphore
            # so later per-o matmuls don't have to (2-wait HW limit).
            ps_dummy = psum_pool.tile([128, 128], fp32, tag="ps")
            nc.tensor.matmul(
                ps_dummy[:],
                lhsT=slot[0:INTER_F, 0:128],
                rhs=slot[0:INTER_F, 0:128],
                start=True,
                stop=True,
            )

            # ---- stage 1b: per-o lhsT [33, 128] = T_ext_o^T @ x_ext ----
            # row c<32: M[i,o,c]; row 32: ones (from x_ext's ones row).
            lhsT_tiles = []
            for o in range(O_PER_CORE):
                ps_o = psum_pool.tile([GW, B], fp32, tag="ps")
                for kk in range(KK):
                    nc.tensor.matmul(
                        ps_o[:],
                        lhsT=te_tiles[kk][:, o * GW : (o + 1) * GW],
                        rhs=xe_tiles[kk][:],
                        start=(kk == 0),
                        stop=(kk == KK - 1),
                    )
                lt = const_pool.tile([GW, B], bf16, tag=f"lt{o}")
                nc.vector.tensor_copy(lt[:], ps_o[:])
                lhsT_tiles.append(lt)

            # acc[i, o] = sum_j exp(-l1[i,j,o])
            acc = const_pool.tile([128, O_PER_CORE], fp32, tag="acc")

            # ---- main loop over output features ----
            H = 4  # j-quarters (2 PSUM banks each)
            JH = B // H  # 64 j values per half
            for o in range(O_PER_CORE):
                # row 32 <- vec(-M_o) (j-major flatten of [128, 32])
                nc.gpsimd.dma_start(
                    slot[INTER_F : INTER_F + 1, :],
                    m_neg[:, o * INTER_F : (o + 1) * INTER_F],
                )
                # absorber: tiny PE op reading slot row 32 so the real
                # matmuls don't each need a fresh DMA-queue wait (2-wait HW
                # limit per PE instruction).
                ps_t = psumt_pool.tile([1, 64], fp32, tag="t")
                nc.tensor.matmul(
                    ps_t[:],
                    lhsT=slot[INTER_F : INTER_F + 1, 0:1],
                    rhs=slot[INTER_F : INTER_F + 1, 0:64],
                    start=True,
                    stop=True,
                )
                l1 = work_pool.tile([128, B], fp32, tag=f"l1_{o}")
                for h in range(H):
                    ps_d = psumd_pool.tile([128, JH * INTER_F], fp32, tag="psd")
                    for b in range(JH * INTER_F // 512):
                        nc.tensor.matmul(
                            ps_d[:, b * 512 : (b + 1) * 512],
                            lhsT=lhsT_tiles[o][:],
                            rhs=slot[:, h * JH * INTER_F + b * 512 :][:, :512],
                            start=True,
                            stop=True,
                        )
                    # l1[i, j] = sum_k |D[i, (j,k)]|
                    nc.vector.tensor_reduce(
                        l1[:, h * JH : (h + 1) * JH],
                        ps_d[:].rearrange("p (j k) -> p j k", k=INTER_F),
                        axis=mybir.AxisListType.X,
                        op=mybir.AluOpType.add,
                        apply_absolute_value=True,
                    )
                escr = work_pool.tile([128, B], bf16, tag="escr")
                nc.scalar.activation(
                    escr[:],
                    l1[:],
                    mybir.ActivationFunctionType.Exp,
                    scale=-1.0,
                    accum_out=acc[:, o : o + 1],
                )

            # ---- diagonal correction + store ----
            zcol = const_pool.tile([128, 1], fp32, tag="zcol")
            nc.vector.memset(zcol[:], 0.0)
            dcol = const_pool.tile([128, 1], fp32, tag="dcol")
            nc.scalar.activation(
                dcol[:], zcol[:], mybir.ActivationFunctionType.Exp, scale=-1.0
            )
            obf = const_pool.tile([128, O_PER_CORE], fp32, tag="obf")
            nc.vector.tensor_scalar(
                obf[:],
                acc[:],
                dcol[:, 0:1],
                None,
                op0=mybir.AluOpType.subtract,
            )
            nc.sync.dma_start(ob_out[:], obf[:])

    nc.finalize()
    return nc


def _prep_inputs(x, T):
    import ml_dtypes

    bf16 = ml_dtypes.bfloat16

    # x_ext^T [1152, 128]: x^T, then a ones row, then zero padding
    xe = np.zeros((KE, B), dtype=np.float32)
    xe[:IN_F, :] = x.T
    xe[IN_F, :] = 1.0
    xe = xe.astype(bf16)

    bones = np.zeros((INTER_F, PAIR_COLS), dtype=bf16)
    for k in range(INTER_F):
        bones[k, k::INTER_F] = 1

    in_maps = []
    for c in range(N_CORES):
        # T_ext [1152, 16*33]: per o-group 32 T columns + a one-hot column
        # (row IN_F = 1) that becomes the lhsT ones row.
        te = np.zeros((KE, O_PER_CORE * GW), dtype=np.float32)
        for o in range(O_PER_CORE):
            blk = T[:, c * COLS_PER_CORE + o * INTER_F : c * COLS_PER_CORE + (o + 1) * INTER_F]
            te[:IN_F, o * GW : o * GW + INTER_F] = blk
            te[IN_F, o * GW + INTER_F] = 1.0
        in_maps.append({"xe": xe, "te": te.astype(bf16), "bones": bones})
    return in_maps


def _install_ntff_hook_shim():
    """Register the axon NTFF profile hook (test-only; used when trace=True).

    The boot package ships the ctypes hook but the image's antenv lacks the
    axon_hooks module concourse imports it from; provide it via sys.modules.
    """
    import sys
    import types

    if "antenv.axon_hooks" in sys.modules:
        return
    try:
        sys.path.insert(0, "/root/.axon_site")
        from trn_agent_boot.trn_boot import _ntff_profile_via_ctypes

        so_path = "/opt/axon/libaxon_pjrt.so"
        hook = _ntff_profile_via_ctypes(so_path)
        mod = types.ModuleType("antenv.axon_hooks")
        mod.get_axon_ntff_profile_hook = lambda: hook
        mod.set_axon_ntff_profile_hook = lambda h: None
        sys.modules["antenv.axon_hooks"] = mod
    except Exception as e:  # profiling is best-effort
        print(f"ntff hook shim failed: {e}")


def _run(x, T, trace=False):
    from concourse.bass_utils import run_bass_kernel_spmd

    if trace:
        _install_ntff_hook_shim()

    if "nc" not in _cache:
        _cache["nc"] = _build_bass()
    nc = _cache["nc"]
    in_maps = _prep_inputs(x, T)
    res = run_bass_kernel_spmd(nc, in_maps, list(range(N_CORES)), trace=trace)
    ob = np.concatenate([res.results[c]["ob"] for c in range(N_CORES)], axis=1)
    out = np.concatenate([x.astype(np.float32), ob.astype(np.float32)], axis=1)
    return out, res


def kernel(x, T):
    x = np.asarray(x, dtype=np.float32)
    T = np.asarray(T, dtype=np.float32)
    out, _ = _run(x, T, trace=False)
    return out
